# revision 1
# baseline (speedup 1.0000x reference)
"""Experts-choose MoE MLP kernel for 8 TRN2 NeuronCores — stage-skewed.

Sharding: core = 2*b + half handles batch row b and experts
[4*half, 4*half+4). Each core emits a partial out(4096,2048); host adds the
two halves per batch row.

The per-(expert, 256-token chunk) work is split into 6 stages run as a
software pipeline over the chunk list, so that within one pipeline step
every engine's work only consumes results produced in earlier steps
(keeps TensorE runs dense -> HAM stays at full clock):
  A: indirect gather of x rows (+ S-matrix build off idx/gate)
  B: PE transposes sel -> selT (D onto partitions)
  C: GEMM1 (k=D, fp32r; b1 folded in as k=1 matmul) + exact Gelu
  D: h-space gated dedup (H2T/gg/cnt matmuls) + copies
  E: GEMM2 (k=H; gated b2 as k=1 matmul) + skip-index calc + y copies
  F: accumulate-scatter (CCE add) into out; non-first duplicate rows are
     redirected to per-partition trash rows [T, T+P) of the padded output.
Output buffers are pre-zeroed by the runtime; untouched tokens stay 0.
"""

import threading

import numpy as np

import concourse.mybir as mybir
import concourse.tile as tile
from concourse import bacc
from concourse.bass import IndirectOffsetOnAxis
from concourse.bass_utils import run_bass_kernel_spmd

P = 128
B, T, D, E, C = 4, 4096, 2048, 8, 1024
H = 256
E_LOC = 4
NCB = C // P
NCP = NCB // 2
N_CORES = 8

F32 = mybir.dt.float32
F32R = mybir.dt.float32r
I32 = mybir.dt.int32
AF = mybir.ActivationFunctionType
OP = mybir.AluOpType


def build_kernel():
    nc = bacc.Bacc("TRN2", target_bir_lowering=False, debug=False)

    x = nc.dram_tensor("x", [T, D], F32R, kind="ExternalInput").ap()
    w1t = nc.dram_tensor("w1t", [E_LOC, D, H], F32R, kind="ExternalInput").ap()
    w2t = nc.dram_tensor("w2t", [E_LOC, H, D], F32R, kind="ExternalInput").ap()
    b1 = nc.dram_tensor("b1", [E_LOC, H], F32R, kind="ExternalInput").ap()
    b2 = nc.dram_tensor("b2", [D], F32R, kind="ExternalInput").ap()
    idx = nc.dram_tensor("idx", [E_LOC, P, NCB], I32, kind="ExternalInput").ap()
    gate = nc.dram_tensor("gate", [E_LOC, P, NCB], F32R, kind="ExternalInput").ap()
    ident_d = nc.dram_tensor("ident", [P, P], F32R, kind="ExternalInput").ap()
    lt_d = nc.dram_tensor("lt", [P, P], F32R, kind="ExternalInput").ap()
    ones_d = nc.dram_tensor("ones", [P, P], F32R, kind="ExternalInput").ap()
    # trash[p] = T + p : unique redirect rows for duplicate hits
    trash_d = nc.dram_tensor("trash", [P, 1], F32R, kind="ExternalInput").ap()
    out = nc.dram_tensor("out", [T, D], F32, kind="ExternalOutput").ap()

    steps = [(e, cp) for e in range(E_LOC) for cp in range(NCP)]
    NS = len(steps)

    with tile.TileContext(nc) as tc:
        with (
            tc.tile_pool(name="const", bufs=1) as const,
            tc.tile_pool(name="wts", bufs=2) as wpool,
            tc.tile_pool(name="meta", bufs=2) as mpool,
            tc.tile_pool(name="work", bufs=2) as spool,
            tc.tile_pool(name="selp", bufs=3) as selpool,
            tc.tile_pool(name="psum", bufs=2, space="PSUM") as ppool,
        ):
            ident = const.tile([P, P], F32R, tag="ident", name="ident")
            nc.sync.dma_start(out=ident, in_=ident_d)
            lt = const.tile([P, P], F32R, tag="lt", name="lt")
            nc.sync.dma_start(out=lt, in_=lt_d)
            ones = const.tile([P, P], F32R, tag="ones", name="ones")
            nc.sync.dma_start(out=ones, in_=ones_d)
            b2_row = const.tile([1, D], F32R, tag="b2row", name="b2_row")
            nc.sync.dma_start(out=b2_row, in_=b2[None, :])
            trash = const.tile([P, 1], F32R, tag="trash", name="trash")
            nc.sync.dma_start(out=trash, in_=trash_d)

            meta = {}
            wts = {}
            st = {}  # per-step pipeline state

            def load_expert(e):
                w1_sb = wpool.tile([P, D // P, H], F32R, tag="w1", name="w1_sb")
                nc.sync.dma_start(
                    out=w1_sb, in_=w1t[e].rearrange("(go gi) j -> gi go j", gi=P)
                )
                w2_sb = wpool.tile([P, H // P, D], F32R, tag="w2", name="w2_sb")
                nc.sync.dma_start(
                    out=w2_sb, in_=w2t[e].rearrange("(jo ji) o -> ji jo o", ji=P)
                )
                b1_row = wpool.tile([1, H], F32R, tag="b1", name="b1_row")
                nc.sync.dma_start(out=b1_row, in_=b1[e][None, :])
                wts[e] = (w1_sb, w2_sb, b1_row)

            def load_meta(e):
                idx_sb = mpool.tile([P, NCB], I32, tag="idx", name="idx_sb")
                nc.sync.dma_start(out=idx_sb, in_=idx[e])
                gate_sb = mpool.tile([P, NCB], F32R, tag="gate", name="gate_sb")
                nc.sync.dma_start(out=gate_sb, in_=gate[e])
                meta[e] = (idx_sb, gate_sb)

            def stage_a(s):
                e, cp = steps[s]
                if cp == 0:
                    load_meta(e)
                    load_expert(e)
                idx_sb, gate_sb = meta[e]
                sel = selpool.tile([P, 2, D], F32R, tag="sel", name="sel")
                for hh in range(2):
                    cb = cp * 2 + hh
                    nc.gpsimd.indirect_dma_start(
                        out=sel[:, hh, :],
                        out_offset=None,
                        in_=x,
                        in_offset=IndirectOffsetOnAxis(
                            ap=idx_sb[:, cb : cb + 1], axis=0
                        ),
                    )
                st[s] = {"sel": sel}

            def stage_b(s):
                e, cp = steps[s]
                idx_sb, gate_sb = meta[e]
                d = st[s]
                sel = d["sel"]
                # PE transposes of the gathered rows
                selt = spool.tile([P, D // P, 2 * P], F32R, tag="selt",
                                  name="selt")
                for hh in range(2):
                    for g4 in range(4):
                        pt = ppool.tile([P, 4 * P], F32R, tag="pt", name="pt")
                        for gg_ in range(4):
                            g = g4 * 4 + gg_
                            nc.tensor.transpose(
                                out=pt[:, gg_ * P : (gg_ + 1) * P],
                                in_=sel[:, hh, g * P : (g + 1) * P],
                                identity=ident,
                            )
                        nc.vector.tensor_copy(
                            out=selt[:, g4 * 4 : (g4 + 1) * 4,
                                     hh * P : (hh + 1) * P],
                            in_=pt.rearrange("p (g c) -> p g c", g=4),
                        )
                # selection matrices (independent of sel; only idx/gate)
                idxfs, sgls = [], []
                for hh in range(2):
                    cb = cp * 2 + hh
                    idxf = spool.tile([P, 1], F32R, tag=f"idxf{hh}",
                                      name="idxf", bufs=4)
                    nc.vector.tensor_copy(out=idxf, in_=idx_sb[:, cb : cb + 1])
                    pit = ppool.tile([P, 4 * P], F32R, tag="pt", name="pit")
                    nc.tensor.transpose(
                        out=pit[:, :P],
                        in_=idxf.to_broadcast([P, P]),
                        identity=ident,
                    )
                    idxt = spool.tile([P, P], F32, tag="idxt", name="idxt")
                    nc.vector.tensor_copy(out=idxt, in_=pit[:, :P])
                    sgl = spool.tile([P, 2, P], F32R, tag=f"sgl{hh}",
                                     name="sgl", bufs=3)
                    nc.vector.tensor_tensor(
                        out=sgl[:, 0, :],
                        in0=idxf.to_broadcast([P, P]),
                        in1=idxt,
                        op=OP.is_equal,
                    )
                    nc.vector.tensor_tensor(
                        out=sgl[:, 0, :],
                        in0=sgl[:, 0, :],
                        in1=gate_sb[:, cb : cb + 1].to_broadcast([P, P]),
                        op=OP.mult,
                    )
                    nc.vector.tensor_tensor(
                        out=sgl[:, 1, :],
                        in0=sgl[:, 0, :],
                        in1=lt,
                        op=OP.mult,
                    )
                    idxfs.append(idxf)
                    sgls.append(sgl)
                d.update(selt=selt, idxfs=idxfs, sgls=sgls)

            def stage_c(s):
                e, cp = steps[s]
                w1_sb, _, b1_row = wts[e]
                d = st[s]
                selt = d["selt"]
                hs = []
                for hh in range(2):
                    ph = ppool.tile([P, H], F32, tag="ph", name="ph")
                    for g in range(D // P):
                        nc.tensor.matmul(
                            out=ph,
                            lhsT=selt[:, g, hh * P : (hh + 1) * P],
                            rhs=w1_sb[:, g, :],
                            start=(g == 0),
                            stop=False,
                        )
                    nc.tensor.matmul(
                        out=ph, lhsT=ones[0:1, :], rhs=b1_row,
                        start=False, stop=True,
                    )
                    h_sb = spool.tile([P, H], F32R, tag=f"h{hh}", name="h_sb",
                                      bufs=2)
                    nc.scalar.activation(out=h_sb, in_=ph, func=AF.Gelu)
                    hs.append(h_sb)
                d["hs"] = hs

            def stage_d(s):
                d = st[s]
                ht2s, ggs, ph2s = [], [], []
                for hh in range(2):
                    h_sb = d["hs"][hh]
                    sgl = d["sgls"][hh]
                    # ph2 bank: [0:256]=H2T, rows[0:2] of [256:384]=gg,
                    # [384:386]=dup cnt
                    ph2 = ppool.tile([P, 4 * P], F32, tag="ph2", name="ph2")
                    for jo in range(H // P):
                        nc.tensor.matmul(
                            out=ph2[:, jo * P : (jo + 1) * P],
                            lhsT=h_sb[:, jo * P : (jo + 1) * P],
                            rhs=sgl[:, 0, :],
                            start=True,
                            stop=True,
                        )
                    nc.tensor.matmul(
                        out=ph2[0:2, 2 * P : 3 * P],
                        lhsT=ones[:, 0:2],
                        rhs=sgl[:, 0, :],
                        start=True,
                        stop=True,
                    )
                    nc.tensor.matmul(
                        out=ph2[:, 3 * P : 3 * P + 2],
                        lhsT=sgl[:, 1, :],
                        rhs=ones[:, 0:2],
                        start=True,
                        stop=True,
                    )
                    ht2 = spool.tile([P, H // P, P], F32R, tag=f"ht2{hh}",
                                     name="ht2", bufs=2)
                    for jo in range(H // P):
                        nc.vector.tensor_copy(
                            out=ht2[:, jo, :],
                            in_=ph2[:, jo * P : (jo + 1) * P],
                        )
                    gg_sb = spool.tile([1, P], F32R, tag=f"gg{hh}",
                                       name="gg_sb", bufs=3)
                    nc.vector.tensor_copy(out=gg_sb,
                                          in_=ph2[0:1, 2 * P : 3 * P])
                    ht2s.append(ht2)
                    ggs.append(gg_sb)
                    ph2s.append(ph2)
                d.update(ht2s=ht2s, ggs=ggs, ph2s=ph2s)

            def stage_e(s):
                e, cp = steps[s]
                _, w2_sb, _ = wts[e]
                d = st[s]
                y_out = spool.tile([P, 2, D], F32, tag="y", name="y_out")
                idx_skip = spool.tile([P, 2], I32, tag="idxs", name="idx_skip")
                for hh in range(2):
                    ht2 = d["ht2s"][hh]
                    gg_sb = d["ggs"][hh]
                    ph2 = d["ph2s"][hh]
                    idxf = d["idxfs"][hh]
                    # idx_skip = dup ? trash_row : idx  (m = min(cnt,1))
                    m = spool.tile([P, 1], F32, tag="m", name="m")
                    nc.vector.tensor_scalar(
                        m, ph2[:, 3 * P : 3 * P + 1], 0.0, 1.0,
                        OP.is_gt, OP.mult,
                    )
                    delta = spool.tile([P, 1], F32, tag="delta", name="delta")
                    nc.vector.tensor_scalar_mul(delta, m, float(2**24))
                    nc.vector.tensor_add(out=delta, in0=delta, in1=idxf)
                    nc.vector.tensor_copy(out=idx_skip[:, hh : hh + 1],
                                          in_=delta)

                    for oc in range(D // 512):
                        py = ppool.tile([P, 512], F32, tag="py", name="py")
                        for jo in range(H // P):
                            nc.tensor.matmul(
                                out=py,
                                lhsT=ht2[:, jo, :],
                                rhs=w2_sb[:, jo, oc * 512 : (oc + 1) * 512],
                                start=(jo == 0),
                                stop=False,
                            )
                        nc.tensor.matmul(
                            out=py,
                            lhsT=gg_sb,
                            rhs=b2_row[:, oc * 512 : (oc + 1) * 512],
                            start=False,
                            stop=True,
                        )
                        nc.scalar.copy(
                            out=y_out[:, hh, oc * 512 : (oc + 1) * 512],
                            in_=py,
                        )
                d.update(y_out=y_out, idx_skip=idx_skip)

            def stage_f(s):
                d = st.pop(s)
                for hh in range(2):
                    nc.gpsimd.indirect_dma_start(
                        out=out,
                        out_offset=IndirectOffsetOnAxis(
                            ap=d["idx_skip"][:, hh : hh + 1], axis=0
                        ),
                        in_=d["y_out"][:, hh, :],
                        in_offset=None,
                        compute_op=OP.add,
                        bounds_check=T - 1,
                        oob_is_err=False,
                    )

            stages = [stage_a, stage_b, stage_c, stage_d, stage_e, stage_f]
            for si in range(NS + len(stages) - 1):
                for k, fn in enumerate(stages):
                    s = si - k
                    if 0 <= s < NS:
                        fn(s)
    nc.compile()
    return nc


_CACHE = {}
_CACHE_LOCK = threading.Lock()


def _get_nc():
    with _CACHE_LOCK:
        if "nc" not in _CACHE:
            _CACHE["nc"] = build_kernel()
        return _CACHE["nc"]


def _make_in_maps(x, W1, b1, W2, b2, expert_indices, expert_gate):
    x = np.ascontiguousarray(x, dtype=np.float32)
    W1 = np.asarray(W1, dtype=np.float32)
    b1 = np.asarray(b1, dtype=np.float32)
    W2 = np.asarray(W2, dtype=np.float32)
    b2 = np.ascontiguousarray(b2, dtype=np.float32)
    idx = np.asarray(expert_indices, dtype=np.int32)
    gate = np.asarray(expert_gate, dtype=np.float32)

    ident = np.eye(P, dtype=np.float32)
    lt = np.triu(np.ones((P, P), dtype=np.float32), 1)
    ones = np.ones((P, P), dtype=np.float32)
    trash = (T + np.arange(P, dtype=np.float32))[:, None]

    in_maps = []
    for core in range(N_CORES):
        b, half = divmod(core, 2)
        es = slice(half * E_LOC, half * E_LOC + E_LOC)
        idx_t = np.ascontiguousarray(
            idx[b, es].reshape(E_LOC, NCB, P).transpose(0, 2, 1)
        )
        gate_t = np.ascontiguousarray(
            gate[b, es].reshape(E_LOC, NCB, P).transpose(0, 2, 1)
        )
        in_maps.append(
            {
                "x": np.ascontiguousarray(x[b]),
                "w1t": np.ascontiguousarray(W1[es].transpose(0, 2, 1)),
                "w2t": np.ascontiguousarray(W2[es].transpose(0, 2, 1)),
                "b1": np.ascontiguousarray(b1[es]),
                "b2": b2,
                "idx": idx_t,
                "gate": gate_t,
                "ident": ident,
                "lt": lt,
                "ones": ones,
                "trash": trash,
            }
        )
    return in_maps


def kernel(x, W1, b1, W2, b2, expert_indices, expert_gate, num_tokens, *,
           _trace=False, _trace_kwargs=None):
    assert int(num_tokens) == T
    nc = _get_nc()
    in_maps = _make_in_maps(x, W1, b1, W2, b2, expert_indices, expert_gate)
    res = run_bass_kernel_spmd(
        nc,
        in_maps,
        core_ids=list(range(N_CORES)),
        trace=_trace,
        **(_trace_kwargs or {}),
    )
    outs = [r["out"] for r in res.results]
    full = np.empty((B, T, D), dtype=np.float32)
    for b in range(B):
        np.add(outs[2 * b], outs[2 * b + 1], out=full[b])
    if _trace:
        kernel.last_results = res
    return full



# revision 5
# speedup vs baseline: 1.2702x; 1.2702x over previous
"""Experts-choose MoE MLP kernel for 8 TRN2 NeuronCores — stage-skewed, bf16.

Sharding: core = 2*b + half handles batch row b and experts
[4*half, 4*half+4). Each core emits a partial out(4096,2048) in bf16; host
adds the two halves per batch row in fp32.

The per-(expert, 256-token chunk) work is split into 6 stages run as a
software pipeline over the chunk list, so that within one pipeline step
every engine's work only consumes results produced in earlier steps
(keeps TensorE runs dense -> HAM stays at full clock):
  A: indirect gather of x rows (bf16) (+ S-matrix build off idx/gate)
  B: PE transposes sel -> selT (D onto partitions), bf16
  C: GEMM1 (k=D, bf16; b1 folded in as k=1 matmul) + exact Gelu
  D: h-space gated dedup (H2T/gg/cnt matmuls, bf16) + copies
  E: GEMM2 (k=H, bf16; gated b2 as k=1 matmul) + skip-index calc + y copies
  F: accumulate-scatter (CCE bf16 add) into out; non-first duplicate rows
     are redirected to index 2^24 (> bounds_check) and silently dropped.
Output buffers are pre-zeroed by the runtime; untouched tokens stay 0.

The idx compare path (idxf/idxt/is_equal, skip-index math) stays fp32 so
token ids up to 4095 stay exact; everything data-sized is bf16 (halves
DMA bytes, enables FWL weight loads, and 1 cycle/row PE everywhere).
"""

import threading

import ml_dtypes
import numpy as np

import concourse.mybir as mybir
import concourse.tile as tile
from concourse import bacc
from concourse.bass import IndirectOffsetOnAxis
from concourse.bass_utils import run_bass_kernel_spmd

P = 128
B, T, D, E, C = 4, 4096, 2048, 8, 1024
H = 256
E_LOC = 4
NCB = C // P
NCP = NCB // 2
N_CORES = 8

F32 = mybir.dt.float32
F32R = mybir.dt.float32r
BF16 = mybir.dt.bfloat16
I32 = mybir.dt.int32
AF = mybir.ActivationFunctionType
OP = mybir.AluOpType

NPBF16 = ml_dtypes.bfloat16


def build_kernel():
    nc = bacc.Bacc("TRN2", target_bir_lowering=False, debug=False)

    x = nc.dram_tensor("x", [T, D], BF16, kind="ExternalInput").ap()
    w1t = nc.dram_tensor("w1t", [E_LOC, D, H], BF16, kind="ExternalInput").ap()
    w2t = nc.dram_tensor("w2t", [E_LOC, H, D], BF16, kind="ExternalInput").ap()
    b1 = nc.dram_tensor("b1", [E_LOC, H], BF16, kind="ExternalInput").ap()
    b2 = nc.dram_tensor("b2", [D], BF16, kind="ExternalInput").ap()
    idx = nc.dram_tensor("idx", [E_LOC, P, NCB], I32, kind="ExternalInput").ap()
    gate = nc.dram_tensor("gate", [E_LOC, P, NCB], BF16,
                          kind="ExternalInput").ap()
    ident_d = nc.dram_tensor("ident", [P, P], BF16, kind="ExternalInput").ap()
    identr_d = nc.dram_tensor("identr", [P, P], F32R,
                              kind="ExternalInput").ap()
    lt_d = nc.dram_tensor("lt", [P, P], BF16, kind="ExternalInput").ap()
    ones_d = nc.dram_tensor("ones", [P, P], BF16, kind="ExternalInput").ap()
    out = nc.dram_tensor("out", [T, D], BF16, kind="ExternalOutput").ap()

    steps = [(e, cp) for e in range(E_LOC) for cp in range(NCP)]
    NS = len(steps)

    with tile.TileContext(nc) as tc:
        with (
            tc.tile_pool(name="const", bufs=1) as const,
            tc.tile_pool(name="wts", bufs=2) as wpool,
            tc.tile_pool(name="meta", bufs=2) as mpool,
            tc.tile_pool(name="work", bufs=2) as spool,
            tc.tile_pool(name="selp", bufs=3) as selpool,
            tc.tile_pool(name="psum", bufs=2, space="PSUM") as ppool,
        ):
            ident = const.tile([P, P], BF16, tag="ident", name="ident")
            nc.sync.dma_start(out=ident, in_=ident_d)
            identr = const.tile([P, P], F32R, tag="identr", name="identr")
            nc.sync.dma_start(out=identr, in_=identr_d)
            lt = const.tile([P, P], BF16, tag="lt", name="lt")
            nc.sync.dma_start(out=lt, in_=lt_d)
            ones = const.tile([P, P], BF16, tag="ones", name="ones")
            nc.sync.dma_start(out=ones, in_=ones_d)
            b2_row = const.tile([1, D], BF16, tag="b2row", name="b2_row")
            nc.sync.dma_start(out=b2_row, in_=b2[None, :])

            meta = {}
            wts = {}
            st = {}  # per-step pipeline state

            def load_expert(e):
                w1_sb = wpool.tile([P, D // P, H], BF16, tag="w1", name="w1_sb")
                nc.sync.dma_start(
                    out=w1_sb, in_=w1t[e].rearrange("(go gi) j -> gi go j", gi=P)
                )
                w2_sb = wpool.tile([P, H // P, D], BF16, tag="w2", name="w2_sb")
                nc.sync.dma_start(
                    out=w2_sb, in_=w2t[e].rearrange("(jo ji) o -> ji jo o", ji=P)
                )
                b1_row = wpool.tile([1, H], BF16, tag="b1", name="b1_row")
                nc.sync.dma_start(out=b1_row, in_=b1[e][None, :])
                wts[e] = (w1_sb, w2_sb, b1_row)

            def load_meta(e):
                idx_sb = mpool.tile([P, NCB], I32, tag="idx", name="idx_sb")
                nc.sync.dma_start(out=idx_sb, in_=idx[e])
                gate_sb = mpool.tile([P, NCB], BF16, tag="gate", name="gate_sb")
                nc.sync.dma_start(out=gate_sb, in_=gate[e])
                meta[e] = (idx_sb, gate_sb)

            def stage_a(s):
                e, cp = steps[s]
                if cp == 0:
                    load_meta(e)
                    load_expert(e)
                idx_sb, gate_sb = meta[e]
                sel = selpool.tile([P, 2, D], BF16, tag="sel", name="sel")
                for hh in range(2):
                    cb = cp * 2 + hh
                    nc.gpsimd.indirect_dma_start(
                        out=sel[:, hh, :],
                        out_offset=None,
                        in_=x,
                        in_offset=IndirectOffsetOnAxis(
                            ap=idx_sb[:, cb : cb + 1], axis=0
                        ),
                    )
                st[s] = {"sel": sel}

            def stage_b(s):
                e, cp = steps[s]
                idx_sb, gate_sb = meta[e]
                d = st[s]
                sel = d["sel"]
                # PE transposes of the gathered rows (bf16, 1 cyc/row)
                selt = spool.tile([P, D // P, 2 * P], BF16, tag="selt",
                                  name="selt")
                for hh in range(2):
                    for g4 in range(4):
                        pt = ppool.tile([P, 4 * P], BF16, tag="pt", name="pt")
                        for gg_ in range(4):
                            g = g4 * 4 + gg_
                            nc.tensor.transpose(
                                out=pt[:, gg_ * P : (gg_ + 1) * P],
                                in_=sel[:, hh, g * P : (g + 1) * P],
                                identity=ident,
                            )
                        nc.scalar.copy(
                            out=selt[:, g4 * 4 : (g4 + 1) * 4,
                                     hh * P : (hh + 1) * P],
                            in_=pt.rearrange("p (g c) -> p g c", g=4),
                        )
                # selection matrices (independent of sel; only idx/gate).
                # idx compare path stays fp32: token ids must stay exact.
                idxfs, sgls = [], []
                for hh in range(2):
                    cb = cp * 2 + hh
                    idxf = spool.tile([P, 1], F32R, tag=f"idxf{hh}",
                                      name="idxf", bufs=4)
                    nc.vector.tensor_copy(out=idxf, in_=idx_sb[:, cb : cb + 1])
                    pit = ppool.tile([P, P], F32R, tag="pit", name="pit",
                                     bufs=1)
                    nc.tensor.transpose(
                        out=pit,
                        in_=idxf.to_broadcast([P, P]),
                        identity=identr,
                    )
                    idxt = spool.tile([P, P], F32, tag="idxt", name="idxt")
                    nc.vector.tensor_copy(out=idxt, in_=pit)
                    sgl = spool.tile([P, 2, P], BF16, tag=f"sgl{hh}",
                                     name="sgl", bufs=3)
                    nc.vector.tensor_tensor(
                        out=sgl[:, 0, :],
                        in0=idxf.to_broadcast([P, P]),
                        in1=idxt,
                        op=OP.is_equal,
                    )
                    nc.vector.tensor_tensor(
                        out=sgl[:, 0, :],
                        in0=sgl[:, 0, :],
                        in1=gate_sb[:, cb : cb + 1].to_broadcast([P, P]),
                        op=OP.mult,
                    )
                    nc.vector.tensor_tensor(
                        out=sgl[:, 1, :],
                        in0=sgl[:, 0, :],
                        in1=lt,
                        op=OP.mult,
                    )
                    idxfs.append(idxf)
                    sgls.append(sgl)
                d.update(selt=selt, idxfs=idxfs, sgls=sgls)

            def stage_c(s):
                e, cp = steps[s]
                w1_sb, _, b1_row = wts[e]
                d = st[s]
                selt = d["selt"]
                hs = []
                for hh in range(2):
                    ph = ppool.tile([P, H], F32, tag="ph", name="ph", bufs=1)
                    for g in range(D // P):
                        nc.tensor.matmul(
                            out=ph,
                            lhsT=selt[:, g, hh * P : (hh + 1) * P],
                            rhs=w1_sb[:, g, :],
                            start=(g == 0),
                            stop=False,
                        )
                    nc.tensor.matmul(
                        out=ph, lhsT=ones[0:1, :], rhs=b1_row,
                        start=False, stop=True,
                    )
                    h_sb = spool.tile([P, H], BF16, tag=f"h{hh}", name="h_sb",
                                      bufs=2)
                    nc.scalar.activation(out=h_sb, in_=ph, func=AF.Gelu)
                    hs.append(h_sb)
                d["hs"] = hs

            def stage_d(s):
                d = st[s]
                ht2s, ggs, ph2s = [], [], []
                for hh in range(2):
                    h_sb = d["hs"][hh]
                    sgl = d["sgls"][hh]
                    # ph2 bank: [0:256]=H2T, rows[0:2] of [256:384]=gg,
                    # [384:386]=dup cnt
                    ph2 = ppool.tile([P, 4 * P], F32, tag="ph2", name="ph2")
                    for jo in range(H // P):
                        nc.tensor.matmul(
                            out=ph2[:, jo * P : (jo + 1) * P],
                            lhsT=h_sb[:, jo * P : (jo + 1) * P],
                            rhs=sgl[:, 0, :],
                            start=True,
                            stop=True,
                        )
                    nc.tensor.matmul(
                        out=ph2[0:2, 2 * P : 3 * P],
                        lhsT=ones[:, 0:2],
                        rhs=sgl[:, 0, :],
                        start=True,
                        stop=True,
                    )
                    nc.tensor.matmul(
                        out=ph2[:, 3 * P : 3 * P + 2],
                        lhsT=sgl[:, 1, :],
                        rhs=ones[:, 0:2],
                        start=True,
                        stop=True,
                    )
                    ht2 = spool.tile([P, H // P, P], BF16, tag=f"ht2{hh}",
                                     name="ht2", bufs=2)
                    for jo in range(H // P):
                        nc.vector.tensor_copy(
                            out=ht2[:, jo, :],
                            in_=ph2[:, jo * P : (jo + 1) * P],
                        )
                    gg_sb = spool.tile([1, P], BF16, tag=f"gg{hh}",
                                       name="gg_sb", bufs=3)
                    nc.vector.tensor_copy(out=gg_sb,
                                          in_=ph2[0:1, 2 * P : 3 * P])
                    ht2s.append(ht2)
                    ggs.append(gg_sb)
                    ph2s.append(ph2)
                d.update(ht2s=ht2s, ggs=ggs, ph2s=ph2s)

            def stage_e(s):
                e, cp = steps[s]
                _, w2_sb, _ = wts[e]
                d = st[s]
                y_out = spool.tile([P, 2, D], BF16, tag="y", name="y_out")
                idx_skip = spool.tile([P, 2], I32, tag="idxs", name="idx_skip")
                for hh in range(2):
                    ht2 = d["ht2s"][hh]
                    gg_sb = d["ggs"][hh]
                    ph2 = d["ph2s"][hh]
                    idxf = d["idxfs"][hh]
                    # idx_skip = dup ? 2^24 + idx (dropped by bounds_check)
                    m = spool.tile([P, 1], F32, tag="m", name="m")
                    nc.vector.tensor_scalar(
                        m, ph2[:, 3 * P : 3 * P + 1], 0.0, 1.0,
                        OP.is_gt, OP.mult,
                    )
                    delta = spool.tile([P, 1], F32, tag="delta", name="delta")
                    nc.vector.tensor_scalar_mul(delta, m, float(2**24))
                    nc.vector.tensor_add(out=delta, in0=delta, in1=idxf)
                    nc.vector.tensor_copy(out=idx_skip[:, hh : hh + 1],
                                          in_=delta)

                    for oc in range(D // 512):
                        py = ppool.tile([P, 512], F32, tag="py", name="py")
                        for jo in range(H // P):
                            nc.tensor.matmul(
                                out=py,
                                lhsT=ht2[:, jo, :],
                                rhs=w2_sb[:, jo, oc * 512 : (oc + 1) * 512],
                                start=(jo == 0),
                                stop=False,
                            )
                        nc.tensor.matmul(
                            out=py,
                            lhsT=gg_sb,
                            rhs=b2_row[:, oc * 512 : (oc + 1) * 512],
                            start=False,
                            stop=True,
                        )
                        cp_eng = nc.scalar if oc == 0 else nc.vector
                        if oc == 0:
                            nc.scalar.copy(
                                out=y_out[:, hh, oc * 512 : (oc + 1) * 512],
                                in_=py,
                            )
                        else:
                            nc.vector.tensor_copy(
                                out=y_out[:, hh, oc * 512 : (oc + 1) * 512],
                                in_=py,
                            )
                d.update(y_out=y_out, idx_skip=idx_skip)

            def stage_f(s):
                d = st.pop(s)
                for hh in range(2):
                    nc.gpsimd.indirect_dma_start(
                        out=out,
                        out_offset=IndirectOffsetOnAxis(
                            ap=d["idx_skip"][:, hh : hh + 1], axis=0
                        ),
                        in_=d["y_out"][:, hh, :],
                        in_offset=None,
                        compute_op=OP.add,
                        bounds_check=T - 1,
                        oob_is_err=False,
                    )

            stages = [stage_a, stage_b, stage_c, stage_d, stage_e, stage_f]
            for si in range(NS + len(stages) - 1):
                for k, fn in enumerate(stages):
                    s = si - k
                    if 0 <= s < NS:
                        fn(s)
    nc.compile()
    return nc


_CACHE = {}
_CACHE_LOCK = threading.Lock()


def _get_nc():
    with _CACHE_LOCK:
        if "nc" not in _CACHE:
            _CACHE["nc"] = build_kernel()
        return _CACHE["nc"]


def _make_in_maps(x, W1, b1, W2, b2, expert_indices, expert_gate):
    x = np.ascontiguousarray(x, dtype=np.float32).astype(NPBF16)
    W1 = np.asarray(W1, dtype=np.float32)
    b1 = np.asarray(b1, dtype=np.float32).astype(NPBF16)
    W2 = np.asarray(W2, dtype=np.float32)
    b2 = np.ascontiguousarray(b2, dtype=np.float32).astype(NPBF16)
    idx = np.asarray(expert_indices, dtype=np.int32)
    gate = np.asarray(expert_gate, dtype=np.float32).astype(NPBF16)

    ident = np.eye(P, dtype=np.float32)
    lt = np.triu(np.ones((P, P), dtype=np.float32), 1)
    ones = np.ones((P, P), dtype=np.float32)

    in_maps = []
    for core in range(N_CORES):
        b, half = divmod(core, 2)
        es = slice(half * E_LOC, half * E_LOC + E_LOC)
        idx_t = np.ascontiguousarray(
            idx[b, es].reshape(E_LOC, NCB, P).transpose(0, 2, 1)
        )
        gate_t = np.ascontiguousarray(
            gate[b, es].reshape(E_LOC, NCB, P).transpose(0, 2, 1)
        )
        in_maps.append(
            {
                "x": np.ascontiguousarray(x[b]),
                "w1t": np.ascontiguousarray(
                    W1[es].transpose(0, 2, 1).astype(NPBF16)),
                "w2t": np.ascontiguousarray(
                    W2[es].transpose(0, 2, 1).astype(NPBF16)),
                "b1": np.ascontiguousarray(b1[es]),
                "b2": b2,
                "idx": idx_t,
                "gate": gate_t,
                "ident": ident.astype(NPBF16),
                "identr": ident,
                "lt": lt.astype(NPBF16),
                "ones": ones.astype(NPBF16),
            }
        )
    return in_maps


def kernel(x, W1, b1, W2, b2, expert_indices, expert_gate, num_tokens, *,
           _trace=False, _trace_kwargs=None):
    assert int(num_tokens) == T
    nc = _get_nc()
    in_maps = _make_in_maps(x, W1, b1, W2, b2, expert_indices, expert_gate)
    res = run_bass_kernel_spmd(
        nc,
        in_maps,
        core_ids=list(range(N_CORES)),
        trace=_trace,
        **(_trace_kwargs or {}),
    )
    outs = [r["out"] for r in res.results]
    full = np.empty((B, T, D), dtype=np.float32)
    for b in range(B):
        np.add(outs[2 * b].astype(np.float32),
               outs[2 * b + 1].astype(np.float32), out=full[b])
    if _trace:
        kernel.last_results = res
    return full


# revision 15
# speedup vs baseline: 1.4420x; 1.1353x over previous
"""Experts-choose MoE MLP kernel for 8 TRN2 NeuronCores — stage-skewed, bf16.

Sharding: core = 2*b + half handles batch row b and experts
[4*half, 4*half+4). Each core emits a partial out(4096,2048) in bf16; host
adds the two halves per batch row in fp32.

The per-(expert, 256-token chunk) work is split into 6 stages run as a
software pipeline over the chunk list, so that within one pipeline step
every engine's work only consumes results produced in earlier steps
(keeps TensorE runs dense -> HAM stays at full clock):
  A: indirect gather of x rows (bf16) (+ S-matrix build off idx/gate)
  B: PE transposes sel -> selT (D onto partitions), bf16
  C: GEMM1 (k=D, bf16; b1 folded in as k=1 matmul) + exact Gelu
  D: h-space gated dedup (H2T/gg/cnt matmuls, bf16) + copies
  E: GEMM2 (k=H, bf16; gated b2 as k=1 matmul) + skip-index calc + y copies
  F: accumulate-scatter (CCE bf16 add) into out; non-first duplicate rows
     are redirected to index 2^24 (> bounds_check) and silently dropped.
Output buffers are pre-zeroed by the runtime; untouched tokens stay 0.

The idx compare path (idxf/idxt/is_equal, skip-index math) stays fp32 so
token ids up to 4095 stay exact; everything data-sized is bf16 (halves
DMA bytes, enables FWL weight loads, and 1 cycle/row PE everywhere).
"""

import threading

import ml_dtypes
import numpy as np

import concourse.mybir as mybir
import concourse.tile as tile
from concourse import bacc
from concourse.bass import IndirectOffsetOnAxis
from concourse.bass_utils import run_bass_kernel_spmd

P = 128
B, T, D, E, C = 4, 4096, 2048, 8, 1024
H = 256
E_LOC = 4
NCB = C // P
NCP = NCB // 2
N_CORES = 8

F32 = mybir.dt.float32
F32R = mybir.dt.float32r
BF16 = mybir.dt.bfloat16
I32 = mybir.dt.int32
I16 = mybir.dt.int16
AF = mybir.ActivationFunctionType
OP = mybir.AluOpType

NPBF16 = ml_dtypes.bfloat16


def build_kernel():
    nc = bacc.Bacc("TRN2", target_bir_lowering=False, debug=False)

    x = nc.dram_tensor("x", [T, D], BF16, kind="ExternalInput").ap()
    w1t = nc.dram_tensor("w1t", [E_LOC, D, H], BF16, kind="ExternalInput").ap()
    w2t = nc.dram_tensor("w2t", [E_LOC, H, D], BF16, kind="ExternalInput").ap()
    b1 = nc.dram_tensor("b1", [E_LOC, H], BF16, kind="ExternalInput").ap()
    b2 = nc.dram_tensor("b2", [D], BF16, kind="ExternalInput").ap()
    idx = nc.dram_tensor("idx", [E_LOC, P, NCB], I32, kind="ExternalInput").ap()
    # idx as int16: column form (for broadcast lhs of the compare) and
    # pre-transposed row-replicated form (kills the on-chip PE transpose)
    idxc16 = nc.dram_tensor("idxc16", [E_LOC, P, NCB], I16,
                            kind="ExternalInput").ap()
    idxt16 = nc.dram_tensor("idxt16", [E_LOC, P, NCB * P], I16,
                            kind="ExternalInput").ap()
    gate = nc.dram_tensor("gate", [E_LOC, P, NCB], BF16,
                          kind="ExternalInput").ap()
    ident_d = nc.dram_tensor("ident", [P, P], BF16, kind="ExternalInput").ap()
    lt_d = nc.dram_tensor("lt", [P, P], BF16, kind="ExternalInput").ap()
    ones_d = nc.dram_tensor("ones", [P, P], BF16, kind="ExternalInput").ap()
    out = nc.dram_tensor("out", [T, D], BF16, kind="ExternalOutput").ap()

    steps = [(e, cp) for e in range(E_LOC) for cp in range(NCP)]
    NS = len(steps)

    with tile.TileContext(nc) as tc:
        with (
            tc.tile_pool(name="const", bufs=1) as const,
            tc.tile_pool(name="wts", bufs=2) as wpool,
            tc.tile_pool(name="meta", bufs=2) as mpool,
            tc.tile_pool(name="work", bufs=2) as spool,
            tc.tile_pool(name="selp", bufs=3) as selpool,
            tc.tile_pool(name="psum", bufs=2, space="PSUM") as ppool,
        ):
            ident = const.tile([P, P], BF16, tag="ident", name="ident")
            nc.sync.dma_start(out=ident, in_=ident_d)
            lt = const.tile([P, P], BF16, tag="lt", name="lt")
            nc.sync.dma_start(out=lt, in_=lt_d)
            ones = const.tile([P, P], BF16, tag="ones", name="ones")
            nc.sync.dma_start(out=ones, in_=ones_d)
            b2_row = const.tile([1, D], BF16, tag="b2row", name="b2_row")
            nc.sync.dma_start(out=b2_row, in_=b2[None, :])

            meta = {}
            wts = {}
            st = {}  # per-step pipeline state

            def load_expert(e):
                w1_sb = wpool.tile([P, D // P, H], BF16, tag="w1", name="w1_sb")
                nc.sync.dma_start(
                    out=w1_sb, in_=w1t[e].rearrange("(go gi) j -> gi go j", gi=P)
                )
                w2_sb = wpool.tile([P, H // P, D], BF16, tag="w2", name="w2_sb")
                nc.sync.dma_start(
                    out=w2_sb, in_=w2t[e].rearrange("(jo ji) o -> ji jo o", ji=P)
                )
                b1_row = wpool.tile([1, H], BF16, tag="b1", name="b1_row")
                nc.sync.dma_start(out=b1_row, in_=b1[e][None, :])
                wts[e] = (w1_sb, w2_sb, b1_row)

            def load_meta(e):
                idx_sb = mpool.tile([P, NCB], I32, tag="idx", name="idx_sb")
                nc.sync.dma_start(out=idx_sb, in_=idx[e])
                idxc_sb = mpool.tile([P, NCB], I16, tag="idxc", name="idxc_sb")
                nc.sync.dma_start(out=idxc_sb, in_=idxc16[e])
                idxt_sb = mpool.tile([P, NCB, P], I16, tag="idxt",
                                     name="idxt_sb")
                nc.sync.dma_start(
                    out=idxt_sb,
                    in_=idxt16[e].rearrange("p (c q) -> p c q", q=P),
                )
                gate_sb = mpool.tile([P, NCB], BF16, tag="gate", name="gate_sb")
                nc.sync.dma_start(out=gate_sb, in_=gate[e])
                meta[e] = (idx_sb, idxc_sb, idxt_sb, gate_sb)

            def stage_a(s):
                e, cp = steps[s]
                if cp == 0:
                    load_meta(e)
                    load_expert(e)
                idx_sb, idxc_sb, idxt_sb, gate_sb = meta[e]
                sel = selpool.tile([P, 2, D], BF16, tag="sel", name="sel")
                for hh in range(2):
                    cb = cp * 2 + hh
                    nc.gpsimd.indirect_dma_start(
                        out=sel[:, hh, :],
                        out_offset=None,
                        in_=x,
                        in_offset=IndirectOffsetOnAxis(
                            ap=idx_sb[:, cb : cb + 1], axis=0
                        ),
                    )
                st[s] = {"sel": sel}

            def stage_b(s):
                e, cp = steps[s]
                idx_sb, idxc_sb, idxt_sb, gate_sb = meta[e]
                d = st[s]
                sel = d["sel"]
                # PE transposes of the gathered rows (bf16, 1 cyc/row);
                # PSUM->SBUF copies alternate ACT/DVE to split the load
                selt = spool.tile([P, D // P, 2 * P], BF16, tag="selt",
                                  name="selt")
                for hh in range(2):
                    for g4 in range(4):
                        pt = ppool.tile([P, 4 * P], BF16, tag="pt", name="pt")
                        for gg_ in range(4):
                            g = g4 * 4 + gg_
                            nc.tensor.transpose(
                                out=pt[:, gg_ * P : (gg_ + 1) * P],
                                in_=sel[:, hh, g * P : (g + 1) * P],
                                identity=ident,
                            )
                        dst = selt[:, g4 * 4 : (g4 + 1) * 4,
                                   hh * P : (hh + 1) * P]
                        src = pt.rearrange("p (g c) -> p g c", g=4)
                        if g4 % 2 == 0:
                            nc.scalar.copy(out=dst, in_=src)
                        else:
                            nc.vector.tensor_copy(out=dst, in_=src)
                # selection matrices (independent of sel; only idx/gate).
                # compare runs on int16 host-pretransposed idx (exact).
                idxfs, sgls = [], []
                for hh in range(2):
                    cb = cp * 2 + hh
                    idxf = spool.tile([P, 1], F32R, tag=f"idxf{hh}",
                                      name="idxf", bufs=4)
                    nc.vector.tensor_copy(out=idxf, in_=idx_sb[:, cb : cb + 1])
                    sgl = spool.tile([P, 2, P], BF16, tag=f"sgl{hh}",
                                     name="sgl", bufs=3)
                    nc.vector.tensor_tensor(
                        out=sgl[:, 0, :],
                        in0=idxc_sb[:, cb : cb + 1].to_broadcast([P, P]),
                        in1=idxt_sb[:, cb, :],
                        op=OP.is_equal,
                    )
                    nc.vector.tensor_tensor(
                        out=sgl[:, 0, :],
                        in0=sgl[:, 0, :],
                        in1=gate_sb[:, cb : cb + 1].to_broadcast([P, P]),
                        op=OP.mult,
                    )
                    nc.vector.tensor_tensor(
                        out=sgl[:, 1, :],
                        in0=sgl[:, 0, :],
                        in1=lt,
                        op=OP.mult,
                    )
                    idxfs.append(idxf)
                    sgls.append(sgl)
                d.update(selt=selt, idxfs=idxfs, sgls=sgls)

            def stage_c(s):
                e, cp = steps[s]
                w1_sb, _, b1_row = wts[e]
                d = st[s]
                selt = d["selt"]
                hs = []
                for hh in range(2):
                    ph = ppool.tile([P, H], F32, tag="ph", name="ph")
                    for g in range(D // P):
                        nc.tensor.matmul(
                            out=ph,
                            lhsT=selt[:, g, hh * P : (hh + 1) * P],
                            rhs=w1_sb[:, g, :],
                            start=(g == 0),
                            stop=False,
                        )
                    nc.tensor.matmul(
                        out=ph, lhsT=ones[0:1, :], rhs=b1_row,
                        start=False, stop=True,
                    )
                    h_sb = spool.tile([P, H], BF16, tag=f"h{hh}", name="h_sb",
                                      bufs=2)
                    nc.scalar.activation(out=h_sb, in_=ph, func=AF.Gelu)
                    hs.append(h_sb)
                d["hs"] = hs

            def stage_d(s):
                d = st[s]
                ht2s, ggs, ph2s = [], [], []
                for hh in range(2):
                    h_sb = d["hs"][hh]
                    sgl = d["sgls"][hh]
                    # ph2 bank: [0:256]=H2T, rows[0:2] of [256:384]=gg,
                    # [384:386]=dup cnt
                    ph2 = ppool.tile([P, 4 * P], F32, tag="ph2", name="ph2")
                    for jo in range(H // P):
                        nc.tensor.matmul(
                            out=ph2[:, jo * P : (jo + 1) * P],
                            lhsT=h_sb[:, jo * P : (jo + 1) * P],
                            rhs=sgl[:, 0, :],
                            start=True,
                            stop=True,
                        )
                    nc.tensor.matmul(
                        out=ph2[0:2, 2 * P : 3 * P],
                        lhsT=ones[:, 0:2],
                        rhs=sgl[:, 0, :],
                        start=True,
                        stop=True,
                    )
                    nc.tensor.matmul(
                        out=ph2[:, 3 * P : 3 * P + 2],
                        lhsT=sgl[:, 1, :],
                        rhs=ones[:, 0:2],
                        start=True,
                        stop=True,
                    )
                    ht2 = spool.tile([P, H // P, P], BF16, tag=f"ht2{hh}",
                                     name="ht2", bufs=2)
                    for jo in range(H // P):
                        nc.vector.tensor_copy(
                            out=ht2[:, jo, :],
                            in_=ph2[:, jo * P : (jo + 1) * P],
                        )
                    gg_sb = spool.tile([1, P], BF16, tag=f"gg{hh}",
                                       name="gg_sb", bufs=3)
                    nc.vector.tensor_copy(out=gg_sb,
                                          in_=ph2[0:1, 2 * P : 3 * P])
                    ht2s.append(ht2)
                    ggs.append(gg_sb)
                    ph2s.append(ph2)
                d.update(ht2s=ht2s, ggs=ggs, ph2s=ph2s)

            def stage_e(s):
                e, cp = steps[s]
                _, w2_sb, _ = wts[e]
                d = st[s]
                y_out = spool.tile([P, 2, D], BF16, tag="y", name="y_out")
                idx_skip = spool.tile([P, 2], I32, tag="idxs", name="idx_skip")
                for hh in range(2):
                    ht2 = d["ht2s"][hh]
                    gg_sb = d["ggs"][hh]
                    ph2 = d["ph2s"][hh]
                    idxf = d["idxfs"][hh]
                    # idx_skip = dup ? 2^24 + idx (dropped by bounds_check)
                    m = spool.tile([P, 1], F32, tag="m", name="m")
                    nc.vector.tensor_scalar(
                        m, ph2[:, 3 * P : 3 * P + 1], 0.0, 1.0,
                        OP.is_gt, OP.mult,
                    )
                    delta = spool.tile([P, 1], F32, tag="delta", name="delta")
                    nc.vector.tensor_scalar_mul(delta, m, float(2**24))
                    nc.vector.tensor_add(out=delta, in0=delta, in1=idxf)
                    nc.vector.tensor_copy(out=idx_skip[:, hh : hh + 1],
                                          in_=delta)

                    for oc in range(D // 512):
                        py = ppool.tile([P, 512], F32, tag="py", name="py")
                        for jo in range(H // P):
                            nc.tensor.matmul(
                                out=py,
                                lhsT=ht2[:, jo, :],
                                rhs=w2_sb[:, jo, oc * 512 : (oc + 1) * 512],
                                start=(jo == 0),
                                stop=False,
                            )
                        nc.tensor.matmul(
                            out=py,
                            lhsT=gg_sb,
                            rhs=b2_row[:, oc * 512 : (oc + 1) * 512],
                            start=False,
                            stop=True,
                        )
                        cp_eng = nc.scalar if oc == 0 else nc.vector
                        if oc == 0:
                            nc.scalar.copy(
                                out=y_out[:, hh, oc * 512 : (oc + 1) * 512],
                                in_=py,
                            )
                        else:
                            nc.vector.tensor_copy(
                                out=y_out[:, hh, oc * 512 : (oc + 1) * 512],
                                in_=py,
                            )
                d.update(y_out=y_out, idx_skip=idx_skip)

            def stage_f(s):
                d = st.pop(s)
                for hh in range(2):
                    nc.gpsimd.indirect_dma_start(
                        out=out,
                        out_offset=IndirectOffsetOnAxis(
                            ap=d["idx_skip"][:, hh : hh + 1], axis=0
                        ),
                        in_=d["y_out"][:, hh, :],
                        in_offset=None,
                        compute_op=OP.add,
                        bounds_check=T - 1,
                        oob_is_err=False,
                    )

            stages = [stage_a, stage_b, stage_c, stage_d, stage_e, stage_f]
            for si in range(NS + len(stages) - 1):
                for k, fn in enumerate(stages):
                    s = si - k
                    if 0 <= s < NS:
                        fn(s)
    nc.compile()
    return nc


_CACHE = {}
_CACHE_LOCK = threading.Lock()


def _get_nc():
    with _CACHE_LOCK:
        if "nc" not in _CACHE:
            _CACHE["nc"] = build_kernel()
        return _CACHE["nc"]


def _make_in_maps(x, W1, b1, W2, b2, expert_indices, expert_gate):
    x = np.ascontiguousarray(x, dtype=np.float32).astype(NPBF16)
    W1 = np.asarray(W1, dtype=np.float32)
    b1 = np.asarray(b1, dtype=np.float32).astype(NPBF16)
    W2 = np.asarray(W2, dtype=np.float32)
    b2 = np.ascontiguousarray(b2, dtype=np.float32).astype(NPBF16)
    idx = np.asarray(expert_indices, dtype=np.int32)
    gate = np.asarray(expert_gate, dtype=np.float32).astype(NPBF16)

    ident = np.eye(P, dtype=np.float32)
    lt = np.triu(np.ones((P, P), dtype=np.float32), 1)
    ones = np.ones((P, P), dtype=np.float32)

    in_maps = []
    for core in range(N_CORES):
        b, half = divmod(core, 2)
        es = slice(half * E_LOC, half * E_LOC + E_LOC)
        idx_t = np.ascontiguousarray(
            idx[b, es].reshape(E_LOC, NCB, P).transpose(0, 2, 1)
        )
        # int16 column form [E,P,NCB] and row-replicated transposed form
        # [E, P, NCB*P]: idxt16[e, p, cb*P + q] = idx[e, cb*P + q] for all p
        idxc16_a = idx_t.astype(np.int16)
        rows = idx[b, es].reshape(E_LOC, 1, NCB * P).astype(np.int16)
        idxt16_a = np.ascontiguousarray(
            np.broadcast_to(rows, (E_LOC, P, NCB * P))
        )
        gate_t = np.ascontiguousarray(
            gate[b, es].reshape(E_LOC, NCB, P).transpose(0, 2, 1)
        )
        in_maps.append(
            {
                "x": np.ascontiguousarray(x[b]),
                "w1t": np.ascontiguousarray(
                    W1[es].transpose(0, 2, 1).astype(NPBF16)),
                "w2t": np.ascontiguousarray(
                    W2[es].transpose(0, 2, 1).astype(NPBF16)),
                "b1": np.ascontiguousarray(b1[es]),
                "b2": b2,
                "idx": idx_t,
                "idxc16": idxc16_a,
                "idxt16": idxt16_a,
                "gate": gate_t,
                "ident": ident.astype(NPBF16),
                "lt": lt.astype(NPBF16),
                "ones": ones.astype(NPBF16),
            }
        )
    return in_maps


def kernel(x, W1, b1, W2, b2, expert_indices, expert_gate, num_tokens, *,
           _trace=False, _trace_kwargs=None):
    assert int(num_tokens) == T
    nc = _get_nc()
    in_maps = _make_in_maps(x, W1, b1, W2, b2, expert_indices, expert_gate)
    res = run_bass_kernel_spmd(
        nc,
        in_maps,
        core_ids=list(range(N_CORES)),
        trace=_trace,
        **(_trace_kwargs or {}),
    )
    outs = [r["out"] for r in res.results]
    full = np.empty((B, T, D), dtype=np.float32)
    for b in range(B):
        np.add(outs[2 * b].astype(np.float32),
               outs[2 * b + 1].astype(np.float32), out=full[b])
    if _trace:
        kernel.last_results = res
    return full
